# revision 33
# baseline (speedup 1.0000x reference)
"""Trainium2 Bass kernel for nn_ExpMatch (gnn_message_passing).

Data-parallel over batch B=256 across 8 NeuronCores (32 rows/core).
Embedding tables (img_features 50000x512, meta_embed 50000x64) are
replicated per core; row gathers are done on-device.

Per-core layout for the heavy path stage:
  (b,p) rows: 32*64 = 2048 = MEGAS(4) x 128 partitions x G(4) groups.
  mega i, partition q=(b_lo*64+p), group g  <->  b_local = i*8 + g*2 + b_lo

The meta gather (32768 rows x 256B per core) is descriptor-generation
bound on the GpSimd Q7 (~9 ns/idx on a single SWDGE queue pair). To
parallelize descgen, the gather is split into 16 slices (one per
(mega, g) group, 2048+128pad idxs each) issued round-robin on the 4
SWDGE queues; queue q runs on Q7 core pair (2q, 2q+1), so up to 4
descgens proceed concurrently, and each slice's doorbell fires early
so the SDMA drain overlaps later descgen.
"""

import sys

import numpy as np

for _p in ("/opt/trn_rl_repo",):
    if _p not in sys.path:
        sys.path.insert(0, _p)

import concourse.bass as bass
import concourse.bacc as bacc
import concourse.tile as tile
from concourse import mybir
from concourse.bass_utils import run_bass_kernel_spmd
from concourse.masks import make_identity

# Problem constants (hardcoded per the harness contract)
B, P, L, D = 256, 64, 16, 64
N_ITEM, IMG_F, META = 50000, 512, 50000
NCORES = 8
BL = B // NCORES            # 32 batch rows per core
MEGAS, G = 4, 4             # 2048 (b,p) rows = MEGAS * 128 * G
GL = G * L                  # lookups per partition per mega (64)
NSLICE = MEGAS * G          # 16 gather slices, one per (mega, g)
MID = 25000                 # midpoint shift so row ids fit signed int16
SL_IDX = L * 128            # real idxs per slice (2048)
SL_PAD = SL_IDX + 128       # plus one pad chunk of valid dummies (2176)
SL_W = SL_PAD // 16         # wrapped idx free dim per slice (136)
IMG_N = 192                 # img gather idxs: 64 real + 128 pad
IMG_W = IMG_N // 16         # wrapped img idx free dim (12)
F32 = mybir.dt.float32
I32 = mybir.dt.int32
I16 = mybir.dt.int16
ALU = mybir.AluOpType
AXT = mybir.AxisListType
ACT = mybir.ActivationFunctionType


def build_nc():
    nc = bacc.Bacc(num_swdge_queues=4)

    ids_w = nc.dram_tensor("ids_w", [128, IMG_W], I16, kind="ExternalInput")
    idx_d = nc.dram_tensor("idx_d", [128, NSLICE * SL_W], I16, kind="ExternalInput")
    mask_d = nc.dram_tensor("mask_d", [128, MEGAS * GL], F32, kind="ExternalInput")
    img = nc.dram_tensor("img", [N_ITEM, IMG_F], F32, kind="ExternalInput")
    meta = nc.dram_tensor("meta", [META, D], F32, kind="ExternalInput")
    imgw = nc.dram_tensor("imgw", [D, IMG_F], F32, kind="ExternalInput")
    imgb = nc.dram_tensor("imgb", [D], F32, kind="ExternalInput")
    w1 = nc.dram_tensor("w1", [D], F32, kind="ExternalInput")
    p1w = nc.dram_tensor("p1w", [D], F32, kind="ExternalInput")
    p1b = nc.dram_tensor("p1b", [1], F32, kind="ExternalInput")
    p2w = nc.dram_tensor("p2w", [D], F32, kind="ExternalInput")
    p2b = nc.dram_tensor("p2b", [1], F32, kind="ExternalInput")
    out_d = nc.dram_tensor("out", [BL, 1], F32, kind="ExternalOutput")
    v_dram = nc.dram_tensor("v_scratch", [BL, D], F32)
    pooled_dram = nc.dram_tensor("pooled_scratch", [BL, D], F32)

    with tile.TileContext(nc) as tc:
        with (
            tc.tile_pool(name="singles", bufs=1) as sg,
            tc.tile_pool(name="work", bufs=3) as wk,
            tc.tile_pool(name="psum", bufs=1, space=bass.MemorySpace.PSUM) as pp,
        ):
            # ---- constants ----
            id128 = sg.tile([128, 128], F32)
            make_identity(nc, id128[:])
            lones = sg.tile([128, 2], F32)
            nc.vector.memset(lones[:], 0.0)
            nc.vector.memset(lones[0:64, 0:1], 1.0)
            nc.vector.memset(lones[64:128, 1:2], 1.0)

            # ---- index loads first so gathers start ASAP ----
            idx_all = sg.tile([128, NSLICE * SL_W], I16)
            nc.sync.dma_start(out=idx_all[:], in_=idx_d[:])
            ids_sb = sg.tile([128, IMG_W], I16)
            nc.sync.dma_start(out=ids_sb[:], in_=ids_w[:])
            msk_all = sg.tile([128, MEGAS * GL], F32)
            nc.sync.dma_start(out=msk_all[:], in_=mask_d[:])

            # ---- img feature rows via a small dma_gather (2KB rows;
            # ~2us descgen, lands early so the img branch's vector ops
            # run before the first mega's data arrives) ----
            G_t = sg.tile([128, 2 * IMG_F], F32)
            nc.gpsimd.dma_gather(
                G_t[:].rearrange("p (c f) -> p c f", f=IMG_F),
                img[MID:, :],
                ids_sb[:],
                IMG_N,
                IMG_N,
                IMG_F,
                single_packet=False,
                queue_num=0,
            )
            G_sb = G_t[0:2 * BL, 0:IMG_F]
            gs_tiles = []
            for s in range(NSLICE):
                gs = wk.tile([128, (L + 1) * D], F32, tag=f"gs{s % G}", bufs=3)
                nc.gpsimd.dma_gather(
                    gs[:].rearrange("p (c d) -> p c d", d=D),
                    meta[MID:, :],
                    idx_all[:, s * SL_W:(s + 1) * SL_W],
                    SL_PAD,
                    SL_PAD,
                    D,
                    single_packet=False,
                    queue_num=(s + 1) % 4,
                )
                gs_tiles.append(gs)

            # ---- img feature branch: linear + l2norm ----
            W_sb = sg.tile([D, IMG_F], F32)
            nc.sync.dma_start(out=W_sb[:], in_=imgw[:])
            GT = sg.tile([128, 4, 2 * BL], F32)
            WT = sg.tile([128, 4, D], F32)
            ps_a = pp.tile([128, 4, 2 * BL], F32)
            ps_b = pp.tile([128, 4, D], F32)
            for k in range(4):
                nc.tensor.transpose(
                    out=ps_a[:, k, :],
                    in_=G_sb[:, k * 128:(k + 1) * 128],
                    identity=id128[0:64, 0:64],
                )
                nc.tensor.transpose(
                    out=ps_b[:, k, :],
                    in_=W_sb[:, k * 128:(k + 1) * 128],
                    identity=id128[0:64, 0:64],
                )
            nc.vector.tensor_copy(GT[:], ps_a[:])
            nc.vector.tensor_copy(WT[:], ps_b[:])

            ps_f = pp.tile([2 * BL, D], F32)
            for k in range(4):
                nc.tensor.matmul(
                    out=ps_f[:],
                    lhsT=GT[:, k, :],
                    rhs=WT[:, k, :],
                    start=(k == 0),
                    stop=(k == 3),
                )

            bias_sb = sg.tile([2 * BL, D], F32)
            nc.sync.dma_start(
                out=bias_sb[:],
                in_=imgb[:].unsqueeze(0).to_broadcast([2 * BL, D]),
            )
            F_sb = sg.tile([2 * BL, D], F32)
            nc.vector.tensor_add(F_sb[:], ps_f[:], bias_sb[:])

            scr = sg.tile([2 * BL, D], F32)
            ssq = sg.tile([2 * BL, 1], F32)
            nc.vector.tensor_mul(scr[:], F_sb[:], F_sb[:])
            nc.vector.reduce_sum(out=ssq[:], in_=scr[:], axis=AXT.X)
            nc.scalar.sqrt(ssq[:], ssq[:])
            nc.vector.tensor_scalar_max(ssq[:], ssq[:], 1e-12)
            nc.vector.reciprocal(ssq[:], ssq[:])
            QR = sg.tile([2 * BL, D], F32)
            nc.vector.tensor_scalar_mul(QR[:], F_sb[:], ssq[:])

            # realign res rows onto partitions 0:31
            R2 = sg.tile([BL, D], F32)
            nc.sync.dma_start(out=R2[:], in_=QR[BL:2 * BL, :])
            uim = sg.tile([BL, D], F32)
            nc.vector.tensor_mul(uim[:], QR[0:BL, :], R2[:])
            uis = sg.tile([BL, D], F32)
            nc.vector.tensor_sub(uis[:], QR[0:BL, :], R2[:])

            w1b = sg.tile([BL, D], F32)
            nc.sync.dma_start(
                out=w1b[:], in_=w1[:].unsqueeze(0).to_broadcast([BL, D])
            )
            v_sb = sg.tile([BL, D], F32)
            nc.vector.scalar_tensor_tensor(
                out=v_sb[:], in0=uis[:], scalar=-1.0, in1=w1b[:],
                op0=ALU.mult, op1=ALU.mult,
            )
            nc.sync.dma_start(out=v_dram[:], in_=v_sb[:])

            # v broadcast to the (b_lo,p)-partition layout for all megas
            V_sb = sg.tile([128, MEGAS, G, D], F32)
            for i in range(MEGAS):
                for b_lo in range(2):
                    v_src = bass.AP(
                        tensor=v_dram[:].tensor,
                        offset=(i * 2 * G + b_lo) * D,
                        ap=[[0, 64], [2 * D, G], [1, D]],
                    )
                    nc.sync.dma_start(
                        out=V_sb[b_lo * 64:(b_lo + 1) * 64, i, :, :], in_=v_src
                    )

            # ---- per-mega: slice norms, pair-sum, 2/3/4-order products,
            # path_res, logits. Program order is mega-major so the in-order
            # DVE stream for mega i never waits on mega i+1's gather data.
            wcol = sg.tile([128, MEGAS * G], F32)
            prs = []
            for i in range(MEGAS):
                # row l2norm scales for all 4 slices, batched (mask folded in)
                Am = wk.tile([128, GL * D], F32, tag="Am", bufs=2)
                for g in range(G):
                    nc.scalar.square(
                        Am[:, g * L * D:(g + 1) * L * D],
                        gs_tiles[i * G + g][:, 0:L * D],
                    )
                ssm = wk.tile([128, GL], F32, tag="ssm", bufs=2)
                nc.vector.reduce_sum(
                    out=ssm[:], in_=Am[:].rearrange("p (n d) -> p n d", d=D),
                    axis=AXT.X,
                )
                nc.scalar.sqrt(ssm[:], ssm[:])
                nc.vector.tensor_scalar_max(ssm[:], ssm[:], 1e-12)
                nc.vector.reciprocal(ssm[:], ssm[:])
                sclm = wk.tile([128, GL], F32, tag="sclm", bufs=2)
                nc.vector.tensor_mul(
                    sclm[:], ssm[:], msk_all[:, i * GL:(i + 1) * GL]
                )
                # pe = row * scale (in place over gs, real chunks only)
                for g in range(G):
                    gs3 = gs_tiles[i * G + g][:, 0:L * D].rearrange(
                        "p (n d) -> p n d", d=D
                    )
                    nc.vector.tensor_tensor(
                        out=gs3, in0=gs3,
                        in1=sclm[:, g * L:(g + 1) * L].unsqueeze(2)
                        .to_broadcast([128, L, D]),
                        op=ALU.mult,
                    )

                pm = wk.tile([128, G, 8, D], F32, tag="pm", bufs=2)
                for g in range(G):
                    pe5 = gs_tiles[i * G + g][:, 0:L * D].rearrange(
                        "p (j t d) -> p j t d", t=2, d=D
                    )
                    nc.vector.tensor_tensor(
                        out=pm[:, g], in0=pe5[:, :, 0, :], in1=pe5[:, :, 1, :],
                        op=ALU.add,
                    )
                p2 = wk.tile([128, G, 7, D], F32, tag="p2", bufs=2)
                nc.vector.tensor_tensor(
                    out=p2[:], in0=pm[:, :, 0:7, :], in1=pm[:, :, 1:8, :],
                    op=ALU.mult,
                )

                S = wk.tile([128, 3, G, D], F32, tag="S", bufs=2)
                T = wk.tile([128, G, 3, D], F32, tag="T", bufs=2)
                U = wk.tile([128, G, D], F32, tag="U", bufs=2)

                # s2 = sum of 7 i2 products
                nc.vector.tensor_tensor(
                    out=T[:], in0=p2[:, :, 0:3, :], in1=p2[:, :, 3:6, :], op=ALU.add
                )
                nc.vector.tensor_tensor(
                    out=U[:], in0=T[:, :, 0, :], in1=T[:, :, 1, :], op=ALU.add
                )
                nc.vector.tensor_tensor(
                    out=U[:], in0=U[:], in1=p2[:, :, 6, :], op=ALU.add
                )
                nc.vector.tensor_tensor(
                    out=S[:, 0, :, :], in0=U[:], in1=T[:, :, 2, :], op=ALU.add
                )
                # i3 products (in place over p2[...,0:6)) and s3
                nc.vector.tensor_tensor(
                    out=p2[:, :, 0:6, :], in0=p2[:, :, 0:6, :],
                    in1=pm[:, :, 2:8, :], op=ALU.mult,
                )
                nc.vector.tensor_tensor(
                    out=T[:], in0=p2[:, :, 0:3, :], in1=p2[:, :, 3:6, :], op=ALU.add
                )
                nc.vector.tensor_tensor(
                    out=U[:], in0=T[:, :, 0, :], in1=T[:, :, 1, :], op=ALU.add
                )
                nc.vector.tensor_tensor(
                    out=S[:, 1, :, :], in0=U[:], in1=T[:, :, 2, :], op=ALU.add
                )
                # i4 products (in place over p2[...,0:5)) and s4
                nc.vector.tensor_tensor(
                    out=p2[:, :, 0:5, :], in0=p2[:, :, 0:5, :],
                    in1=pm[:, :, 3:8, :], op=ALU.mult,
                )
                nc.vector.tensor_tensor(
                    out=T[:, :, 0:2, :], in0=p2[:, :, 0:2, :],
                    in1=p2[:, :, 2:4, :], op=ALU.add,
                )
                nc.vector.tensor_tensor(
                    out=U[:], in0=T[:, :, 0, :], in1=T[:, :, 1, :], op=ALU.add
                )
                nc.vector.tensor_tensor(
                    out=S[:, 2, :, :], in0=U[:], in1=p2[:, :, 4, :], op=ALU.add
                )

                # r_k = l2norm(s_k)/3, path_res = r1+r2+r3
                SQ = wk.tile([128, 3 * G * D], F32, tag="SQ", bufs=2)
                nc.scalar.square(SQ[:], S[:])
                ssk = wk.tile([128, 3 * G], F32, tag="ssk", bufs=2)
                nc.vector.reduce_sum(
                    out=ssk[:], in_=SQ[:].rearrange("p (n d) -> p n d", d=D),
                    axis=AXT.X,
                )
                # 3*norm via sqrt(9*ss); clamp matches x/(3*max(n,1e-12))
                nc.scalar.activation(ssk[:], ssk[:], ACT.Sqrt, scale=9.0)
                nc.vector.tensor_scalar_max(ssk[:], ssk[:], 3e-12)
                nc.vector.reciprocal(ssk[:], ssk[:])
                invk = ssk[:].rearrange("p (k n) -> p k n", k=3)

                pr = sg.tile([128, G, D], F32, tag=f"pr{i}")
                nc.vector.tensor_tensor(
                    out=pr[:], in0=S[:, 0, :, :],
                    in1=invk[:, 0, :].unsqueeze(2).to_broadcast([128, G, D]),
                    op=ALU.mult,
                )
                nc.vector.tensor_tensor(
                    out=U[:], in0=S[:, 1, :, :],
                    in1=invk[:, 1, :].unsqueeze(2).to_broadcast([128, G, D]),
                    op=ALU.mult,
                )
                nc.vector.tensor_add(pr[:], pr[:], U[:])
                nc.vector.tensor_tensor(
                    out=U[:], in0=S[:, 2, :, :],
                    in1=invk[:, 2, :].unsqueeze(2).to_broadcast([128, G, D]),
                    op=ALU.mult,
                )
                nc.vector.tensor_add(pr[:], pr[:], U[:])
                prs.append(pr)

                # attention logits: wcol[:, i*G+g] = sum_d v[b]*path_res
                AW = wk.tile([128, G, D], F32, tag="AW", bufs=2)
                nc.vector.tensor_mul(AW[:], pr[:], V_sb[:, i])
                nc.vector.reduce_sum(
                    out=wcol[:, i * G:(i + 1) * G], in_=AW[:], axis=AXT.X
                )

            # ---- softmax over p (via two PE transposes) ----
            wT_ps = pp.tile([MEGAS * G, 128], F32)
            nc.tensor.transpose(out=wT_ps[:], in_=wcol[:], identity=id128[:])
            wT = sg.tile([MEGAS * G, 128], F32)
            nc.vector.tensor_copy(wT[:], wT_ps[:])
            wT3 = wT[:].rearrange("c (b p) -> c b p", b=2)
            mx = sg.tile([MEGAS * G, 2], F32)
            nc.vector.reduce_max(out=mx[:], in_=wT3, axis=AXT.X)
            xs = sg.tile([MEGAS * G, 128], F32)
            nc.vector.tensor_tensor(
                out=xs[:].rearrange("c (b p) -> c b p", b=2), in0=wT3,
                in1=mx[:].unsqueeze(2).to_broadcast([MEGAS * G, 2, 64]),
                op=ALU.subtract,
            )
            ex = sg.tile([MEGAS * G, 128], F32)
            nc.scalar.activation(ex[:], xs[:], ACT.Exp, scale=5.0)
            sm = sg.tile([MEGAS * G, 2], F32)
            nc.vector.reduce_sum(
                out=sm[:], in_=ex[:].rearrange("c (b p) -> c b p", b=2), axis=AXT.X
            )
            nc.vector.reciprocal(sm[:], sm[:])
            wf = sg.tile([MEGAS * G, 128], F32)
            nc.vector.tensor_tensor(
                out=wf[:].rearrange("c (b p) -> c b p", b=2),
                in0=ex[:].rearrange("c (b p) -> c b p", b=2),
                in1=sm[:].unsqueeze(2).to_broadcast([MEGAS * G, 2, 64]),
                op=ALU.mult,
            )
            wc2_ps = pp.tile([128, MEGAS * G], F32)
            nc.tensor.transpose(
                out=wc2_ps[:], in_=wf[:], identity=id128[0:MEGAS * G, 0:MEGAS * G]
            )
            wc2 = sg.tile([128, MEGAS * G], F32)
            nc.vector.tensor_copy(wc2[:], wc2_ps[:])

            # ---- weighted pooling over p: one block-ones matmul per mega ----
            for i in range(MEGAS):
                wpr = wk.tile([128, G, D], F32, tag="wpr")
                nc.vector.tensor_tensor(
                    out=wpr[:], in0=prs[i][:],
                    in1=wc2[:, i * G:(i + 1) * G].unsqueeze(2)
                    .to_broadcast([128, G, D]),
                    op=ALU.mult,
                )
                pool_ps = pp.tile([2, G * D], F32)
                nc.tensor.matmul(
                    out=pool_ps[:], lhsT=lones[:],
                    rhs=wpr[:].rearrange("p n d -> p (n d)"),
                    start=True, stop=True,
                )
                pool_sb = wk.tile([2, G * D], F32, tag="pool_sb")
                nc.vector.tensor_copy(pool_sb[:], pool_ps[:])
                nc.sync.dma_start(
                    out=pooled_dram[i * 2 * G:(i + 1) * 2 * G, :]
                    .rearrange("(g b) d -> b g d", g=G),
                    in_=pool_sb[:].rearrange("b (g d) -> b g d", d=D),
                )

            pooled = sg.tile([BL, D], F32)
            nc.sync.dma_start(out=pooled[:], in_=pooled_dram[:])

            # ---- final scores ----
            z = sg.tile([BL, D], F32)
            nc.vector.scalar_tensor_tensor(
                out=z[:], in0=uis[:], scalar=-1.0, in1=pooled[:],
                op0=ALU.mult, op1=ALU.mult,
            )
            p2wb = sg.tile([BL, D], F32)
            nc.sync.dma_start(
                out=p2wb[:], in_=p2w[:].unsqueeze(0).to_broadcast([BL, D])
            )
            p1wb = sg.tile([BL, D], F32)
            nc.sync.dma_start(
                out=p1wb[:], in_=p1w[:].unsqueeze(0).to_broadcast([BL, D])
            )
            p2bb = sg.tile([BL, 1], F32)
            nc.sync.dma_start(
                out=p2bb[:], in_=p2b[:].unsqueeze(0).to_broadcast([BL, 1])
            )
            p1bb = sg.tile([BL, 1], F32)
            nc.sync.dma_start(
                out=p1bb[:], in_=p1b[:].unsqueeze(0).to_broadcast([BL, 1])
            )
            scrA = sg.tile([BL, D], F32)
            s2c = sg.tile([BL, 1], F32)
            nc.vector.tensor_mul(scrA[:], z[:], p2wb[:])
            nc.vector.reduce_sum(out=s2c[:], in_=scrA[:], axis=AXT.X)
            nc.vector.tensor_add(s2c[:], s2c[:], p2bb[:])
            scrB = sg.tile([BL, D], F32)
            s1c = sg.tile([BL, 1], F32)
            nc.vector.tensor_mul(scrB[:], uim[:], p1wb[:])
            nc.vector.reduce_sum(out=s1c[:], in_=scrB[:], axis=AXT.X)
            nc.vector.tensor_add(s1c[:], s1c[:], p1bb[:])
            fin = sg.tile([BL, 1], F32)
            nc.vector.scalar_tensor_tensor(
                out=fin[:], in0=s2c[:], scalar=5.0, in1=s1c[:],
                op0=ALU.mult, op1=ALU.add,
            )
            nc.sync.dma_start(out=out_d[:], in_=fin[:])

    nc.compile()
    return nc


def wrap_img_idx(ids):
    """[64] img row ids -> padded wrapped int16 [128, IMG_W]."""
    flat = np.full(IMG_N, MID, np.int64)
    flat[:2 * BL] = ids
    sh = (flat - MID).astype(np.int16)
    w16 = sh.reshape(-1, 16).T            # [16, IMG_W]
    return np.ascontiguousarray(np.tile(w16, (8, 1)))


def wrap_slice_idx(idx_slice):
    """[128, L] row-major (q, l) -> padded wrapped int16 [128, SL_W].

    Flat gather order i = c*128 + q for chunk c=l; one trailing pad chunk
    of valid dummies (shifted 0) prevents the ucode's trailing-negative
    trim from eating real (MID-shifted negative) indices.
    """
    flat = np.empty(SL_PAD, np.int64)
    flat[:SL_IDX] = idx_slice.T.reshape(-1)
    flat[SL_IDX:] = MID
    sh = (flat - MID).astype(np.int16)
    w16 = sh.reshape(-1, 16).T            # [16, SL_W]
    return np.ascontiguousarray(np.tile(w16, (8, 1)))


def make_in_maps(inputs):
    """Shard full inputs into 8 per-core input maps."""
    qry = np.asarray(inputs["qry_id"]).astype(np.int32)
    res = np.asarray(inputs["res_id"]).astype(np.int32)
    path = np.asarray(inputs["path"]).astype(np.int32)
    mask = np.asarray(inputs["mask"]).astype(np.float32)
    shared = {
        "img": np.ascontiguousarray(np.asarray(inputs["img_features"], np.float32)),
        "meta": np.ascontiguousarray(np.asarray(inputs["meta_embed"], np.float32)),
        "imgw": np.ascontiguousarray(np.asarray(inputs["imageW_w"], np.float32)),
        "imgb": np.ascontiguousarray(np.asarray(inputs["imageW_b"], np.float32)),
        "w1": np.ascontiguousarray(
            np.asarray(inputs["h_att_w"], np.float32)[0, :D]
        ),
        "p1w": np.ascontiguousarray(np.asarray(inputs["predict1_w"], np.float32)[0]),
        "p1b": np.ascontiguousarray(
            np.asarray(inputs["predict1_b"], np.float32).reshape(1)
        ),
        "p2w": np.ascontiguousarray(np.asarray(inputs["predict2_w"], np.float32)[0]),
        "p2b": np.ascontiguousarray(
            np.asarray(inputs["predict2_b"], np.float32).reshape(1)
        ),
    }
    in_maps = []
    for c in range(NCORES):
        s = slice(c * BL, (c + 1) * BL)
        ids_w = wrap_img_idx(np.concatenate([qry[s, 0], res[s, 0]]))
        # mega layout: [i, (b_lo,p), g, l] with b_local = i*8 + g*2 + b_lo
        pr = path[s].reshape(MEGAS, G, 2, P, L)
        idx = np.ascontiguousarray(pr.transpose(0, 2, 3, 1, 4)).reshape(
            MEGAS, 128, G, L
        )
        idx_sl = np.ascontiguousarray(
            np.stack(
                [
                    wrap_slice_idx(idx[i, :, g, :])
                    for i in range(MEGAS)
                    for g in range(G)
                ]
            ).transpose(1, 0, 2).reshape(128, NSLICE * SL_W)
        )
        mr = mask[s].reshape(MEGAS, G, 2, P, L)
        mk = np.ascontiguousarray(
            mr.transpose(2, 3, 0, 1, 4).reshape(128, MEGAS * GL)
        )
        in_maps.append(
            dict(shared, ids_w=ids_w, idx_d=idx_sl, mask_d=mk)
        )
    return in_maps


_NC_CACHE = None


def kernel(**inputs):
    global _NC_CACHE
    if _NC_CACHE is None:
        _NC_CACHE = build_nc()
    in_maps = make_in_maps(inputs)
    res = run_bass_kernel_spmd(_NC_CACHE, in_maps, list(range(NCORES)))
    out = np.concatenate([res.results[c]["out"] for c in range(NCORES)], axis=0)
    return out.astype(np.float32)


if __name__ == "__main__":
    nc = build_nc()
    print("build ok")


# revision 39
# speedup vs baseline: 1.0586x; 1.0586x over previous
"""Trainium2 Bass kernel for nn_ExpMatch (gnn_message_passing).

Data-parallel over batch B=256 across 8 NeuronCores (32 rows/core).
Embedding tables (img_features 50000x512, meta_embed 50000x64) are
replicated per core; row gathers are done on-device.

Per-core layout for the heavy path stage:
  (b,p) rows: 32*64 = 2048 = MEGAS(4) x 128 partitions x G(4) groups.
  mega i, partition q=(b_lo*64+p), group g  <->  b_local = i*8 + g*2 + b_lo

The meta gather (32768 rows x 256B per core) is descriptor-generation
bound on the GpSimd Q7 (~9 ns/idx on a single SWDGE queue pair). To
parallelize descgen, the gather is split into 16 slices (one per
(mega, g) group, 2048+128pad idxs each) issued round-robin on the 4
SWDGE queues; queue q runs on Q7 core pair (2q, 2q+1), so up to 4
descgens proceed concurrently, and each slice's doorbell fires early
so the SDMA drain overlaps later descgen.
"""

import sys

import numpy as np

for _p in ("/opt/trn_rl_repo",):
    if _p not in sys.path:
        sys.path.insert(0, _p)

import concourse.bass as bass
import concourse.bacc as bacc
import concourse.tile as tile
from concourse import mybir
from concourse.bass_utils import run_bass_kernel_spmd
from concourse.masks import make_identity

# Problem constants (hardcoded per the harness contract)
B, P, L, D = 256, 64, 16, 64
N_ITEM, IMG_F, META = 50000, 512, 50000
NCORES = 8
BL = B // NCORES            # 32 batch rows per core
MEGAS, G = 4, 4             # 2048 (b,p) rows = MEGAS * 128 * G
GL = G * L                  # lookups per partition per mega (64)
NSLICE = MEGAS * G          # 16 gather slices, one per (mega, g)
MID = 25000                 # midpoint shift so row ids fit signed int16
SL_IDX = L * 128            # real idxs per slice (2048)
SL_PAD = SL_IDX + 128       # plus one pad chunk of valid dummies (2176)
SL_W = SL_PAD // 16         # wrapped idx free dim per slice (136)
IMG_N = 192                 # img gather idxs: 64 real + 128 pad
IMG_W = IMG_N // 16         # wrapped img idx free dim (12)
F32 = mybir.dt.float32
I32 = mybir.dt.int32
I16 = mybir.dt.int16
ALU = mybir.AluOpType
AXT = mybir.AxisListType
ACT = mybir.ActivationFunctionType


def build_nc():
    nc = bacc.Bacc(num_swdge_queues=4)

    ids_qr = nc.dram_tensor("ids_qr", [2 * BL, 1], I32, kind="ExternalInput")
    idx_d = nc.dram_tensor("idx_d", [128, NSLICE * SL_W], I16, kind="ExternalInput")
    mask_d = nc.dram_tensor("mask_d", [128, MEGAS * GL], F32, kind="ExternalInput")
    img = nc.dram_tensor("img", [N_ITEM, IMG_F], F32, kind="ExternalInput")
    meta = nc.dram_tensor("meta", [META, D], F32, kind="ExternalInput")
    imgw = nc.dram_tensor("imgw", [D, IMG_F], F32, kind="ExternalInput")
    imgb = nc.dram_tensor("imgb", [D], F32, kind="ExternalInput")
    w1 = nc.dram_tensor("w1", [D], F32, kind="ExternalInput")
    p1w = nc.dram_tensor("p1w", [D], F32, kind="ExternalInput")
    p1b = nc.dram_tensor("p1b", [1], F32, kind="ExternalInput")
    p2w = nc.dram_tensor("p2w", [D], F32, kind="ExternalInput")
    p2b = nc.dram_tensor("p2b", [1], F32, kind="ExternalInput")
    out_d = nc.dram_tensor("out", [BL, 1], F32, kind="ExternalOutput")
    v_dram = nc.dram_tensor("v_scratch", [BL, D], F32)
    pooled_dram = nc.dram_tensor("pooled_scratch", [BL, D], F32)

    with tile.TileContext(nc) as tc:
        with (
            tc.tile_pool(name="singles", bufs=1) as sg,
            tc.tile_pool(name="work", bufs=3) as wk,
            tc.tile_pool(name="psum", bufs=1, space=bass.MemorySpace.PSUM) as pp,
        ):
            # ---- constants ----
            id128 = sg.tile([128, 128], F32)
            make_identity(nc, id128[:])
            lones = sg.tile([128, 2], F32)
            nc.vector.memset(lones[:], 0.0)
            nc.vector.memset(lones[0:64, 0:1], 1.0)
            nc.vector.memset(lones[64:128, 1:2], 1.0)

            # ---- img feature branch head: ids + indirect gather (gpsimd
            # first, before descgen floods the pool engine — its DVE chain
            # must run before the meta descgen window to avoid SBUF-port
            # starvation of the in-order DVE stream) ----
            ids_sb = sg.tile([2 * BL, 1], I32)
            nc.sync.dma_start(out=ids_sb[:], in_=ids_qr[:])
            G_sb = sg.tile([2 * BL, IMG_F], F32)
            nc.gpsimd.indirect_dma_start(
                out=G_sb[:],
                out_offset=None,
                in_=img[:],
                in_offset=bass.IndirectOffsetOnAxis(ap=ids_sb[:, :1], axis=0),
            )

            idx_all = sg.tile([128, NSLICE * SL_W], I16)
            nc.sync.dma_start(out=idx_all[:], in_=idx_d[:])
            msk_all = sg.tile([128, MEGAS * GL], F32)
            nc.sync.dma_start(out=msk_all[:], in_=mask_d[:])
            gs_tiles = []
            for s in range(NSLICE):
                gs = wk.tile([128, (L + 1) * D], F32, tag=f"gs{s % G}", bufs=4)
                nc.gpsimd.dma_gather(
                    gs[:].rearrange("p (c d) -> p c d", d=D),
                    meta[MID:, :],
                    idx_all[:, s * SL_W:(s + 1) * SL_W],
                    SL_PAD,
                    SL_PAD,
                    D,
                    single_packet=False,
                    queue_num=(s + 1) % 4,
                )
                gs_tiles.append(gs)

            # ---- img feature branch: linear + l2norm ----
            W_sb = sg.tile([D, IMG_F], F32)
            nc.sync.dma_start(out=W_sb[:], in_=imgw[:])
            GT = sg.tile([128, 4, 2 * BL], F32)
            WT = sg.tile([128, 4, D], F32)
            ps_a = pp.tile([128, 4, 2 * BL], F32)
            ps_b = pp.tile([128, 4, D], F32)
            for k in range(4):
                nc.tensor.transpose(
                    out=ps_a[:, k, :],
                    in_=G_sb[:, k * 128:(k + 1) * 128],
                    identity=id128[0:64, 0:64],
                )
                nc.tensor.transpose(
                    out=ps_b[:, k, :],
                    in_=W_sb[:, k * 128:(k + 1) * 128],
                    identity=id128[0:64, 0:64],
                )
            nc.vector.tensor_copy(GT[:], ps_a[:])
            nc.vector.tensor_copy(WT[:], ps_b[:])

            ps_f = pp.tile([2 * BL, D], F32)
            for k in range(4):
                nc.tensor.matmul(
                    out=ps_f[:],
                    lhsT=GT[:, k, :],
                    rhs=WT[:, k, :],
                    start=(k == 0),
                    stop=(k == 3),
                )

            bias_sb = sg.tile([2 * BL, D], F32)
            nc.sync.dma_start(
                out=bias_sb[:],
                in_=imgb[:].unsqueeze(0).to_broadcast([2 * BL, D]),
            )
            F_sb = sg.tile([2 * BL, D], F32)
            nc.vector.tensor_add(F_sb[:], ps_f[:], bias_sb[:])

            scr = sg.tile([2 * BL, D], F32)
            ssq = sg.tile([2 * BL, 1], F32)
            nc.vector.tensor_mul(scr[:], F_sb[:], F_sb[:])
            nc.vector.reduce_sum(out=ssq[:], in_=scr[:], axis=AXT.X)
            nc.scalar.sqrt(ssq[:], ssq[:])
            nc.vector.tensor_scalar_max(ssq[:], ssq[:], 1e-12)
            nc.vector.reciprocal(ssq[:], ssq[:])
            QR = sg.tile([2 * BL, D], F32)
            nc.vector.tensor_scalar_mul(QR[:], F_sb[:], ssq[:])

            # realign res rows onto partitions 0:31
            R2 = sg.tile([BL, D], F32)
            nc.sync.dma_start(out=R2[:], in_=QR[BL:2 * BL, :])
            uim = sg.tile([BL, D], F32)
            nc.vector.tensor_mul(uim[:], QR[0:BL, :], R2[:])
            uis = sg.tile([BL, D], F32)
            nc.vector.tensor_sub(uis[:], QR[0:BL, :], R2[:])

            w1b = sg.tile([BL, D], F32)
            nc.sync.dma_start(
                out=w1b[:], in_=w1[:].unsqueeze(0).to_broadcast([BL, D])
            )
            v_sb = sg.tile([BL, D], F32)
            nc.vector.scalar_tensor_tensor(
                out=v_sb[:], in0=uis[:], scalar=-1.0, in1=w1b[:],
                op0=ALU.mult, op1=ALU.mult,
            )
            nc.sync.dma_start(out=v_dram[:], in_=v_sb[:])

            # v broadcast to the (b_lo,p)-partition layout for all megas
            V_sb = sg.tile([128, MEGAS, G, D], F32)
            for i in range(MEGAS):
                for b_lo in range(2):
                    v_src = bass.AP(
                        tensor=v_dram[:].tensor,
                        offset=(i * 2 * G + b_lo) * D,
                        ap=[[0, 64], [2 * D, G], [1, D]],
                    )
                    nc.sync.dma_start(
                        out=V_sb[b_lo * 64:(b_lo + 1) * 64, i, :, :], in_=v_src
                    )

            # ---- per-mega: slice norms, pair-sum, 2/3/4-order products,
            # path_res, logits. Program order is mega-major so the in-order
            # DVE stream for mega i never waits on mega i+1's gather data.
            wcol = sg.tile([128, MEGAS * G], F32)
            prs = []
            for i in range(MEGAS):
                # row l2norm scales for all 4 slices, batched (mask folded in)
                Am = wk.tile([128, GL * D], F32, tag="Am", bufs=2)
                for g in range(G):
                    nc.scalar.square(
                        Am[:, g * L * D:(g + 1) * L * D],
                        gs_tiles[i * G + g][:, 0:L * D],
                    )
                ssm = wk.tile([128, GL], F32, tag="ssm", bufs=2)
                nc.vector.reduce_sum(
                    out=ssm[:], in_=Am[:].rearrange("p (n d) -> p n d", d=D),
                    axis=AXT.X,
                )
                nc.scalar.sqrt(ssm[:], ssm[:])
                nc.vector.tensor_scalar_max(ssm[:], ssm[:], 1e-12)
                nc.vector.reciprocal(ssm[:], ssm[:])
                sclm = wk.tile([128, GL], F32, tag="sclm", bufs=2)
                nc.vector.tensor_mul(
                    sclm[:], ssm[:], msk_all[:, i * GL:(i + 1) * GL]
                )
                # pe = row * scale (in place over gs, real chunks only)
                for g in range(G):
                    gs3 = gs_tiles[i * G + g][:, 0:L * D].rearrange(
                        "p (n d) -> p n d", d=D
                    )
                    nc.vector.tensor_tensor(
                        out=gs3, in0=gs3,
                        in1=sclm[:, g * L:(g + 1) * L].unsqueeze(2)
                        .to_broadcast([128, L, D]),
                        op=ALU.mult,
                    )

                pm = wk.tile([128, G, 8, D], F32, tag="pm", bufs=2)
                for g in range(G):
                    pe5 = gs_tiles[i * G + g][:, 0:L * D].rearrange(
                        "p (j t d) -> p j t d", t=2, d=D
                    )
                    nc.vector.tensor_tensor(
                        out=pm[:, g], in0=pe5[:, :, 0, :], in1=pe5[:, :, 1, :],
                        op=ALU.add,
                    )
                p2 = wk.tile([128, G, 7, D], F32, tag="p2", bufs=2)
                nc.vector.tensor_tensor(
                    out=p2[:], in0=pm[:, :, 0:7, :], in1=pm[:, :, 1:8, :],
                    op=ALU.mult,
                )

                S = wk.tile([128, 3, G, D], F32, tag="S", bufs=2)
                T = wk.tile([128, G, 3, D], F32, tag="T", bufs=2)
                U = wk.tile([128, G, D], F32, tag="U", bufs=2)

                # s2 = sum of 7 i2 products
                nc.vector.tensor_tensor(
                    out=T[:], in0=p2[:, :, 0:3, :], in1=p2[:, :, 3:6, :], op=ALU.add
                )
                nc.vector.tensor_tensor(
                    out=U[:], in0=T[:, :, 0, :], in1=T[:, :, 1, :], op=ALU.add
                )
                nc.vector.tensor_tensor(
                    out=U[:], in0=U[:], in1=p2[:, :, 6, :], op=ALU.add
                )
                nc.vector.tensor_tensor(
                    out=S[:, 0, :, :], in0=U[:], in1=T[:, :, 2, :], op=ALU.add
                )
                # i3 products (in place over p2[...,0:6)) and s3
                nc.vector.tensor_tensor(
                    out=p2[:, :, 0:6, :], in0=p2[:, :, 0:6, :],
                    in1=pm[:, :, 2:8, :], op=ALU.mult,
                )
                nc.vector.tensor_tensor(
                    out=T[:], in0=p2[:, :, 0:3, :], in1=p2[:, :, 3:6, :], op=ALU.add
                )
                nc.vector.tensor_tensor(
                    out=U[:], in0=T[:, :, 0, :], in1=T[:, :, 1, :], op=ALU.add
                )
                nc.vector.tensor_tensor(
                    out=S[:, 1, :, :], in0=U[:], in1=T[:, :, 2, :], op=ALU.add
                )
                # i4 products (in place over p2[...,0:5)) and s4
                nc.vector.tensor_tensor(
                    out=p2[:, :, 0:5, :], in0=p2[:, :, 0:5, :],
                    in1=pm[:, :, 3:8, :], op=ALU.mult,
                )
                nc.vector.tensor_tensor(
                    out=T[:, :, 0:2, :], in0=p2[:, :, 0:2, :],
                    in1=p2[:, :, 2:4, :], op=ALU.add,
                )
                nc.vector.tensor_tensor(
                    out=U[:], in0=T[:, :, 0, :], in1=T[:, :, 1, :], op=ALU.add
                )
                nc.vector.tensor_tensor(
                    out=S[:, 2, :, :], in0=U[:], in1=p2[:, :, 4, :], op=ALU.add
                )

                # r_k = l2norm(s_k)/3, path_res = r1+r2+r3
                SQ = wk.tile([128, 3 * G * D], F32, tag="SQ", bufs=2)
                nc.scalar.square(SQ[:], S[:])
                ssk = wk.tile([128, 3 * G], F32, tag="ssk", bufs=2)
                nc.vector.reduce_sum(
                    out=ssk[:], in_=SQ[:].rearrange("p (n d) -> p n d", d=D),
                    axis=AXT.X,
                )
                # 3*norm via sqrt(9*ss); clamp matches x/(3*max(n,1e-12))
                nc.scalar.activation(ssk[:], ssk[:], ACT.Sqrt, scale=9.0)
                nc.vector.tensor_scalar_max(ssk[:], ssk[:], 3e-12)
                nc.vector.reciprocal(ssk[:], ssk[:])
                invk = ssk[:].rearrange("p (k n) -> p k n", k=3)

                pr = sg.tile([128, G, D], F32, tag=f"pr{i}")
                nc.vector.tensor_tensor(
                    out=pr[:], in0=S[:, 0, :, :],
                    in1=invk[:, 0, :].unsqueeze(2).to_broadcast([128, G, D]),
                    op=ALU.mult,
                )
                nc.vector.tensor_tensor(
                    out=U[:], in0=S[:, 1, :, :],
                    in1=invk[:, 1, :].unsqueeze(2).to_broadcast([128, G, D]),
                    op=ALU.mult,
                )
                nc.vector.tensor_add(pr[:], pr[:], U[:])
                nc.vector.tensor_tensor(
                    out=U[:], in0=S[:, 2, :, :],
                    in1=invk[:, 2, :].unsqueeze(2).to_broadcast([128, G, D]),
                    op=ALU.mult,
                )
                nc.vector.tensor_add(pr[:], pr[:], U[:])
                prs.append(pr)

                # attention logits: wcol[:, i*G+g] = sum_d v[b]*path_res
                AW = wk.tile([128, G, D], F32, tag="AW", bufs=2)
                nc.vector.tensor_mul(AW[:], pr[:], V_sb[:, i])
                nc.vector.reduce_sum(
                    out=wcol[:, i * G:(i + 1) * G], in_=AW[:], axis=AXT.X
                )

            # ---- softmax over p (via two PE transposes) ----
            wT_ps = pp.tile([MEGAS * G, 128], F32)
            nc.tensor.transpose(out=wT_ps[:], in_=wcol[:], identity=id128[:])
            wT = sg.tile([MEGAS * G, 128], F32)
            nc.vector.tensor_copy(wT[:], wT_ps[:])
            wT3 = wT[:].rearrange("c (b p) -> c b p", b=2)
            mx = sg.tile([MEGAS * G, 2], F32)
            nc.vector.reduce_max(out=mx[:], in_=wT3, axis=AXT.X)
            xs = sg.tile([MEGAS * G, 128], F32)
            nc.vector.tensor_tensor(
                out=xs[:].rearrange("c (b p) -> c b p", b=2), in0=wT3,
                in1=mx[:].unsqueeze(2).to_broadcast([MEGAS * G, 2, 64]),
                op=ALU.subtract,
            )
            ex = sg.tile([MEGAS * G, 128], F32)
            nc.scalar.activation(ex[:], xs[:], ACT.Exp, scale=5.0)
            sm = sg.tile([MEGAS * G, 2], F32)
            nc.vector.reduce_sum(
                out=sm[:], in_=ex[:].rearrange("c (b p) -> c b p", b=2), axis=AXT.X
            )
            nc.vector.reciprocal(sm[:], sm[:])
            wf = sg.tile([MEGAS * G, 128], F32)
            nc.vector.tensor_tensor(
                out=wf[:].rearrange("c (b p) -> c b p", b=2),
                in0=ex[:].rearrange("c (b p) -> c b p", b=2),
                in1=sm[:].unsqueeze(2).to_broadcast([MEGAS * G, 2, 64]),
                op=ALU.mult,
            )
            wc2_ps = pp.tile([128, MEGAS * G], F32)
            nc.tensor.transpose(
                out=wc2_ps[:], in_=wf[:], identity=id128[0:MEGAS * G, 0:MEGAS * G]
            )
            wc2 = sg.tile([128, MEGAS * G], F32)
            nc.vector.tensor_copy(wc2[:], wc2_ps[:])

            # ---- weighted pooling over p: one block-ones matmul per mega ----
            for i in range(MEGAS):
                wpr = wk.tile([128, G, D], F32, tag="wpr")
                nc.vector.tensor_tensor(
                    out=wpr[:], in0=prs[i][:],
                    in1=wc2[:, i * G:(i + 1) * G].unsqueeze(2)
                    .to_broadcast([128, G, D]),
                    op=ALU.mult,
                )
                pool_ps = pp.tile([2, G * D], F32)
                nc.tensor.matmul(
                    out=pool_ps[:], lhsT=lones[:],
                    rhs=wpr[:].rearrange("p n d -> p (n d)"),
                    start=True, stop=True,
                )
                pool_sb = wk.tile([2, G * D], F32, tag="pool_sb")
                nc.vector.tensor_copy(pool_sb[:], pool_ps[:])
                nc.sync.dma_start(
                    out=pooled_dram[i * 2 * G:(i + 1) * 2 * G, :]
                    .rearrange("(g b) d -> b g d", g=G),
                    in_=pool_sb[:].rearrange("b (g d) -> b g d", d=D),
                )

            pooled = sg.tile([BL, D], F32)
            nc.sync.dma_start(out=pooled[:], in_=pooled_dram[:])

            # ---- final scores ----
            z = sg.tile([BL, D], F32)
            nc.vector.scalar_tensor_tensor(
                out=z[:], in0=uis[:], scalar=-1.0, in1=pooled[:],
                op0=ALU.mult, op1=ALU.mult,
            )
            p2wb = sg.tile([BL, D], F32)
            nc.sync.dma_start(
                out=p2wb[:], in_=p2w[:].unsqueeze(0).to_broadcast([BL, D])
            )
            p1wb = sg.tile([BL, D], F32)
            nc.sync.dma_start(
                out=p1wb[:], in_=p1w[:].unsqueeze(0).to_broadcast([BL, D])
            )
            p2bb = sg.tile([BL, 1], F32)
            nc.sync.dma_start(
                out=p2bb[:], in_=p2b[:].unsqueeze(0).to_broadcast([BL, 1])
            )
            p1bb = sg.tile([BL, 1], F32)
            nc.sync.dma_start(
                out=p1bb[:], in_=p1b[:].unsqueeze(0).to_broadcast([BL, 1])
            )
            scrA = sg.tile([BL, D], F32)
            s2c = sg.tile([BL, 1], F32)
            nc.vector.tensor_mul(scrA[:], z[:], p2wb[:])
            nc.vector.reduce_sum(out=s2c[:], in_=scrA[:], axis=AXT.X)
            nc.vector.tensor_add(s2c[:], s2c[:], p2bb[:])
            scrB = sg.tile([BL, D], F32)
            s1c = sg.tile([BL, 1], F32)
            nc.vector.tensor_mul(scrB[:], uim[:], p1wb[:])
            nc.vector.reduce_sum(out=s1c[:], in_=scrB[:], axis=AXT.X)
            nc.vector.tensor_add(s1c[:], s1c[:], p1bb[:])
            fin = sg.tile([BL, 1], F32)
            nc.vector.scalar_tensor_tensor(
                out=fin[:], in0=s2c[:], scalar=5.0, in1=s1c[:],
                op0=ALU.mult, op1=ALU.add,
            )
            nc.sync.dma_start(out=out_d[:], in_=fin[:])

    nc.compile()
    return nc


def wrap_slice_idx(idx_slice):
    """[128, L] row-major (q, l) -> padded wrapped int16 [128, SL_W].

    Flat gather order i = c*128 + q for chunk c=l; one trailing pad chunk
    of valid dummies (shifted 0) prevents the ucode's trailing-negative
    trim from eating real (MID-shifted negative) indices.
    """
    flat = np.empty(SL_PAD, np.int64)
    flat[:SL_IDX] = idx_slice.T.reshape(-1)
    flat[SL_IDX:] = MID
    sh = (flat - MID).astype(np.int16)
    w16 = sh.reshape(-1, 16).T            # [16, SL_W]
    return np.ascontiguousarray(np.tile(w16, (8, 1)))


def make_in_maps(inputs):
    """Shard full inputs into 8 per-core input maps."""
    qry = np.asarray(inputs["qry_id"]).astype(np.int32)
    res = np.asarray(inputs["res_id"]).astype(np.int32)
    path = np.asarray(inputs["path"]).astype(np.int32)
    mask = np.asarray(inputs["mask"]).astype(np.float32)
    shared = {
        "img": np.ascontiguousarray(np.asarray(inputs["img_features"], np.float32)),
        "meta": np.ascontiguousarray(np.asarray(inputs["meta_embed"], np.float32)),
        "imgw": np.ascontiguousarray(np.asarray(inputs["imageW_w"], np.float32)),
        "imgb": np.ascontiguousarray(np.asarray(inputs["imageW_b"], np.float32)),
        "w1": np.ascontiguousarray(
            np.asarray(inputs["h_att_w"], np.float32)[0, :D]
        ),
        "p1w": np.ascontiguousarray(np.asarray(inputs["predict1_w"], np.float32)[0]),
        "p1b": np.ascontiguousarray(
            np.asarray(inputs["predict1_b"], np.float32).reshape(1)
        ),
        "p2w": np.ascontiguousarray(np.asarray(inputs["predict2_w"], np.float32)[0]),
        "p2b": np.ascontiguousarray(
            np.asarray(inputs["predict2_b"], np.float32).reshape(1)
        ),
    }
    in_maps = []
    for c in range(NCORES):
        s = slice(c * BL, (c + 1) * BL)
        ids_qr = np.concatenate([qry[s, 0], res[s, 0]])[:, None].astype(np.int32)
        # mega layout: [i, (b_lo,p), g, l] with b_local = i*8 + g*2 + b_lo
        pr = path[s].reshape(MEGAS, G, 2, P, L)
        idx = np.ascontiguousarray(pr.transpose(0, 2, 3, 1, 4)).reshape(
            MEGAS, 128, G, L
        )
        idx_sl = np.ascontiguousarray(
            np.stack(
                [
                    wrap_slice_idx(idx[i, :, g, :])
                    for i in range(MEGAS)
                    for g in range(G)
                ]
            ).transpose(1, 0, 2).reshape(128, NSLICE * SL_W)
        )
        mr = mask[s].reshape(MEGAS, G, 2, P, L)
        mk = np.ascontiguousarray(
            mr.transpose(2, 3, 0, 1, 4).reshape(128, MEGAS * GL)
        )
        in_maps.append(
            dict(shared, ids_qr=ids_qr, idx_d=idx_sl, mask_d=mk)
        )
    return in_maps


_NC_CACHE = None


def kernel(**inputs):
    global _NC_CACHE
    if _NC_CACHE is None:
        _NC_CACHE = build_nc()
    in_maps = make_in_maps(inputs)
    res = run_bass_kernel_spmd(_NC_CACHE, in_maps, list(range(NCORES)))
    out = np.concatenate([res.results[c]["out"] for c in range(NCORES)], axis=0)
    return out.astype(np.float32)


if __name__ == "__main__":
    nc = build_nc()
    print("build ok")
